# revision 32
# baseline (speedup 1.0000x reference)
"""Trainium2 Bass kernel for the Luong-attention module.

Shapes (hardcoded): B=64, T=128, S=1024, IN=1024, OUT=1024.
Sharding: data-parallel over batch across 8 NeuronCores (8 batches/core).
All matmuls run in fp16 (fp32 PSUM accumulation).

Mask compaction: the padding mask removes ~half the encoder positions.
The host gathers only unmasked positions (padded with zeros to a fixed
SP, a multiple of 128), the kernel computes scores/softmax/ctx over SP
columns, and the host scatters attn_weights back to the full S grid
(masked positions are exactly 0: zeroed E^T columns give score 0, and
exp(0 - max) underflows to 0 in f16 since the unmasked max >> 0).

Per-core dataflow (contraction dim always on partitions):
  q_projT[i,t]   = sum_o W_attnT[o,i] * QT[o,t]          (once, all 8 batches)
  scores[t,s]    = sum_i q_projT[i,t] * ET[i,s]          (s over SP compact)
  softmax along s (free axis): negmax -> Exp(bias)+accum_out -> reciprocal
  w16[t,s]       = ew * (1/sum)   (ACT, per-partition scale)
  wT[s,t]        = PE-transpose(w16)                     (SP/128 tiles)
  ctx[t,i]       = sum_s wT[s,t].T * E[s,i]   (wT stationary, N=512 streams)
  ctxT[i,t]      = PE-transpose(ctx[t,i])                (8 tiles)
  out[t,o]       = tanh(sum_c catT[c,t] * W_outT[c,o] + b_out)
                   with catT k-tiles = [qT; ctxT], W_out host-packed
                   decoder-half first.

Scheduling: phase 0 runs ko-outer over four PSUM groups so the wq DMA
window is filled with real qproj work; the out-projection accumulates
its two 512-col halves in separate PSUM tiles (tanh of one half never
serializes the other half's matmuls); softmax_back is emitted after the
ctxT-cast block so recip never heads-of-line-blocks the DVE FIFO; input
DMAs are deadline-ordered on the sync ring with triple-buffered et/en.
"""

import numpy as np

import concourse.bass as bass
import concourse.mybir as mybir
import concourse.tile as tile
from concourse import bacc
from concourse.bass_utils import run_bass_kernel_spmd
from concourse.masks import make_identity

F16 = mybir.dt.float16
F32 = mybir.dt.float32

N_CORES = 8
B_LOC = 8          # batches per core
T = 128
S = 1024
IN = 1024
OUT = 1024
C = IN + OUT       # concat dim
KO = OUT // 128    # k-tiles over o
KI = IN // 128     # k-tiles over i
KC = C // 128      # k-tiles over c
TALL = B_LOC * T   # stacked t across local batches
SP_MIN = 640       # compacted encoder length (padded, multiple of 128)
N_WARMUP = 58      # identity matmuls: cover the pre-wq DMA window warm

_CACHED = {}


def _ts(i, sz):
    return slice(i * sz, (i + 1) * sz)


def _chunks(total, sz=512):
    """[(offset, len), ...] covering `total` in <=sz pieces."""
    out = []
    off = 0
    while off < total:
        ln = min(sz, total - off)
        out.append((off, ln))
        off += ln
    return out


def _build_program(with_bias, slots):
    # slots[j] = encoder k-tile count for local batch slot j (the host
    # deals batches into slots sorted by unmasked count, so later slots
    # compile with fewer scores/ctx tiles).
    kspmax = max(slots)
    spmax = kspmax * 128
    nc = bacc.Bacc("TRN2", target_bir_lowering=False, debug=False)

    wq = nc.dram_tensor("wq", [128, KO, 2, IN], F16, kind="ExternalInput")
    et = nc.dram_tensor("et", [B_LOC, 128, KI, spmax], F16, kind="ExternalInput")
    en = nc.dram_tensor("en", [B_LOC, 128, kspmax, IN], F16, kind="ExternalInput")
    # wot is host-packed decoder-half first: [:, 0:KO] = W_out[:, IN:C].T
    # tiles, [:, KO:] = W_out[:, 0:IN].T tiles (earliest consumer first).
    wot = nc.dram_tensor("wot", [128, KC, OUT], F16, kind="ExternalInput")
    bb = nc.dram_tensor("bb", [1, OUT], F16, kind="ExternalInput")
    w_out = nc.dram_tensor("w_out", [B_LOC, T, spmax], F16, kind="ExternalOutput")
    att_out = nc.dram_tensor("att_out", [B_LOC, T, OUT], F16, kind="ExternalOutput")

    with tile.TileContext(nc) as tc:
        with (
            tc.tile_pool(name="const", bufs=1) as const_pool,
            tc.tile_pool(name="inp", bufs=3) as in_pool,
            tc.tile_pool(name="ewp", bufs=2) as ew_pool,
            tc.tile_pool(name="statp", bufs=2) as stat_pool,
            tc.tile_pool(name="xp", bufs=2) as x_pool,
            tc.tile_pool(name="outp", bufs=2) as out_pool,
            tc.tile_pool(name="pssp", bufs=2, space="PSUM") as pss_pool,
            tc.tile_pool(name="pmix", bufs=1, space="PSUM") as pmix_pool,
            tc.tile_pool(name="psop", bufs=1, space="PSUM") as pso_pool,
        ):
            ident = const_pool.tile([128, 128], F16)
            make_identity(nc, ident[:])
            ones = const_pool.tile([1, 128], F16)
            nc.vector.memset(ones[:], 1.0)
            # Pre-load the ACT exp/tanh spline tables during the DMA-bound
            # head instead of on exp(0)'s critical path.
            actwarm = const_pool.tile([1, 128], F32)
            nc.scalar.activation(actwarm[:], ones[:],
                                 mybir.ActivationFunctionType.Exp)
            if with_bias:
                bb_sb = const_pool.tile([1, OUT], F16)
                nc.sync.dma_start(bb_sb[:], bb[:])

            # One trigger per ko delivers both wat[ko] and qt[ko].
            wq_sb = const_pool.tile([128, KO, 2, IN], F16)
            for ko in range(KO):
                nc.sync.dma_start(wq_sb[:, ko, :, :], wq[:, ko, :, :])

            qpt_sb = const_pool.tile([128, KI, TALL], F16)

            et_t, en_t = {}, {}

            def load_et(b):
                sp = slots[b] * 128
                et_sb = in_pool.tile([128, KI, sp], F16, name="et")
                nc.sync.dma_start(et_sb[:], et[b][:, :, :sp])
                et_t[b] = et_sb

            def load_en(b):
                ksp = slots[b]
                en_sb = in_pool.tile([128, ksp, IN], F16, name="en")
                nc.sync.dma_start(en_sb[:], en[b][:, :ksp, :])
                en_t[b] = en_sb

            # ---- Phase 0: q_projT[i, t_all] for all local batches ----
            # ko-outer over four PSUM accumulation groups (mi 0-3) keeps
            # the PE ~fully busy with real work while the wq chunks stream
            # in.  mi 4-7 then run as pure compute.
            psqA = pss_pool.tile([128, TALL], F32, name="psqA", tag="pss")
            psqB = pss_pool.tile([128, TALL], F32, name="psqB", tag="pss")
            psqC = pmix_pool.tile([128, TALL], F32, name="psqC", tag="mix")
            # mi=3 splits across the two out-projection PSUM half tiles.
            pso_a = pso_pool.tile([128, 512], F32, name="pso_a", tag="pso_a")
            pso_b = pso_pool.tile([128, 512], F32, name="pso_b", tag="pso_b")

            # Burn the HAM cold window on identity matmuls into psqA's
            # region while the first weight/query DMAs are in flight
            # (qproj's ko=0 start=True overwrites the junk).
            for _ in range(N_WARMUP):
                nc.tensor.matmul(psqA[:, :128], ident[:], ident[:],
                                 start=True, stop=True)

            head = [psqA, psqB, psqC]
            for ko in range(KO):
                for mi in range(3):
                    for nh in range(TALL // 512):
                        nc.tensor.matmul(
                            head[mi][:, _ts(nh, 512)],
                            wq_sb[:, ko, 0, _ts(mi, 128)],
                            wq_sb[:, ko, 1, _ts(nh, 512)],
                            start=(ko == 0),
                            stop=(ko == KO - 1),
                        )
                for nh, pshalf in ((0, pso_a), (1, pso_b)):
                    nc.tensor.matmul(
                        pshalf[:],
                        wq_sb[:, ko, 0, _ts(3, 128)],
                        wq_sb[:, ko, 1, _ts(nh, 512)],
                        start=(ko == 0),
                        stop=(ko == KO - 1),
                    )
            # mi0/mi1 casts first: they gate the pss buffers mi4/mi5 need.
            for mi in range(3):
                nc.vector.tensor_copy(qpt_sb[:, mi, :], head[mi][:])
            nc.vector.tensor_copy(qpt_sb[:, 3, :512], pso_a[:])
            nc.vector.tensor_copy(qpt_sb[:, 3, 512:], pso_b[:])
            for mi in range(4, KI):
                psq = pss_pool.tile([128, TALL], F32, name="psq", tag="pss")
                for ko in range(KO):
                    for nh in range(TALL // 512):
                        nc.tensor.matmul(
                            psq[:, _ts(nh, 512)],
                            wq_sb[:, ko, 0, _ts(mi, 128)],
                            wq_sb[:, ko, 1, _ts(nh, 512)],
                            start=(ko == 0),
                            stop=(ko == KO - 1),
                        )
                nc.vector.tensor_copy(qpt_sb[:, mi, :], psq[:])

            # Deadline-ordered input stream on the sync ring (single FIFO
            # queue): wq, et0, et1, wot-dec, en0, wot-ctx, then per
            # iteration et(b+2), en(b+1).
            load_et(0)
            load_et(1)
            wot_sb = const_pool.tile([128, KC, OUT], F16)
            nc.sync.dma_start(wot_sb[:, :KO, :], wot[:, :KO, :])
            load_en(0)
            nc.sync.dma_start(wot_sb[:, KO:, :], wot[:, KO:, :])

            def scores_mms(b):
                sp = slots[b] * 128
                et_sb = et_t[b]
                pss = pss_pool.tile([128, sp], F32, name="pss", tag="pss")
                for ki in range(KI):
                    for off, ln in _chunks(sp):
                        nc.tensor.matmul(
                            pss[:, off:off + ln],
                            qpt_sb[:, ki, _ts(b, T)],
                            et_sb[:, ki, off:off + ln],
                            start=(ki == 0),
                            stop=(ki == KI - 1),
                        )
                return pss

            def softmax_front(b, pss):
                sp = slots[b] * 128
                negmx = stat_pool.tile([128, 1], F32, name="negmx")
                nc.vector.reduce_max(
                    negmx[:], pss[:], axis=mybir.AxisListType.X, negate=True
                )
                ew = ew_pool.tile([128, sp], F16, name="ew")
                ssum = stat_pool.tile([128, 1], F32, name="ssum")
                nc.scalar.activation(
                    ew[:],
                    pss[:],
                    mybir.ActivationFunctionType.Exp,
                    bias=negmx[:],
                    scale=1.0,
                    accum_out=ssum[:],
                )
                return ew, ssum

            def softmax_back(b, ew, ssum):
                # Normalize on ACT (per-partition 1/sum scale); the
                # normalized w16 feeds both the w_out DMA and the PE
                # transpose, so ctx needs no downstream rescale.
                sp = slots[b] * 128
                rs = stat_pool.tile([128, 1], F32, name="rs")
                nc.vector.reciprocal(rs[:], ssum[:])
                w16 = ew_pool.tile([128, sp], F16, name="w16")
                nc.scalar.mul(w16[:], ew[:], rs[:])
                nc.scalar.dma_start(w_out[b][:, :sp], w16[:])
                return w16

            def transp_w(b, w16):
                # wT[s, t] via PE transpose; the DVE casts drain during the
                # next scores block.
                ksp = slots[b]
                pst = pmix_pool.tile([128, ksp, T], F16, name="pst", tag="mix")
                for st in range(ksp):
                    nc.tensor.matmul(
                        pst[:, st, :],
                        w16[:, _ts(st, 128)],
                        ident[:],
                        is_transpose=True,
                        start=True,
                        stop=True,
                    )
                wt_sb = x_pool.tile([128, ksp, T], F16, name="wt")
                h = ksp // 2
                nc.vector.tensor_copy(wt_sb[:, :h, :], pst[:, :h, :])
                nc.vector.tensor_copy(wt_sb[:, h:, :], pst[:, h:, :])
                return wt_sb

            def ctx_mms(b, en_sb, wt_sb):
                # ctx[t, i] = sum_s w16[t,s] E[s,i]: wT tiles stationary,
                # E streams at N=512.  Weights are already normalized, so
                # the PSUM->SBUF cast is a plain ACT copy.
                ksp = slots[b]
                psc = pmix_pool.tile([128, IN], F32, name="psc", tag="mix")
                for ks in range(ksp):
                    for nh in range(IN // 512):
                        nc.tensor.matmul(
                            psc[:, _ts(nh, 512)],
                            wt_sb[:, ks, :],
                            en_sb[:, ks, _ts(nh, 512)],
                            start=(ks == 0),
                            stop=(ks == ksp - 1),
                        )
                cx16 = x_pool.tile([128, IN], F16, name="cx16")
                nc.scalar.copy(cx16[:, : IN // 2], psc[:, : IN // 2])
                nc.scalar.copy(cx16[:, IN // 2 :], psc[:, IN // 2 :])
                return cx16

            def transpose_ctx(cx16):
                # ctxT[i, t] via PE transpose of cx16 (cheap: ~60ns/tile);
                # the DVE cast of each half chases the transposes.
                pct = pmix_pool.tile([128, KI, T], F16, name="pct", tag="mix")
                ctxt_sb = x_pool.tile([128, KI, T], F16, name="ctxT")
                for j in range(KI):
                    nc.tensor.matmul(
                        pct[:, j, :],
                        cx16[:, _ts(j, 128)],
                        ident[:],
                        is_transpose=True,
                        start=True,
                        stop=True,
                    )
                    if j == KI // 2 - 1:
                        nc.vector.tensor_copy(
                            ctxt_sb[:, : KI // 2, :], pct[:, : KI // 2, :])
                nc.vector.tensor_copy(
                    ctxt_sb[:, KI // 2 :, :], pct[:, KI // 2 :, :])
                return ctxt_sb

            def mm_qt(ps, b, kq, nh, start):
                nc.tensor.matmul(
                    ps[:],
                    wq_sb[:, kq, 1, _ts(b, T)],
                    wot_sb[:, kq, _ts(nh, 512)],
                    start=start,
                    stop=False,
                )

            def mm_bias(ps, nh):
                nc.tensor.matmul(
                    ps[:],
                    ones[:1, :],
                    bb_sb[:1, _ts(nh, 512)],
                    start=True,
                    stop=False,
                )

            def mm_ctx_all(ps, ctxt_sb, nh):
                for kc in range(KI):
                    nc.tensor.matmul(
                        ps[:],
                        ctxt_sb[:, kc, :],
                        wot_sb[:, KO + kc, _ts(nh, 512)],
                        start=False,
                        stop=(kc == KI - 1),
                    )

            def out_part1(b, cx16):
                # out[t, o] = tanh(catT.T @ W_outT + b_out).  The two
                # 512-col halves accumulate in separate PSUM tiles so the
                # tanh reads never serialize the other half's matmuls.
                ps_a = pso_pool.tile([128, 512], F32, name="pso_a", tag="pso_a")
                ps_b = pso_pool.tile([128, 512], F32, name="pso_b", tag="pso_b")
                halves = ((0, ps_a), (1, ps_b))
                if with_bias:
                    for nh, ps in halves:
                        mm_bias(ps, nh)
                for kq in range(KO // 2):
                    for nh, ps in halves:
                        mm_qt(ps, b, kq, nh, start=(not with_bias and kq == 0))
                ctxt_sb = transpose_ctx(cx16)
                return ps_a, ps_b, ctxt_sb

            def out_part2(b, ps_a, ps_b, ctxt_sb):
                halves = ((0, ps_a), (1, ps_b))
                for kq in range(KO // 2, KO):
                    for nh, ps in halves:
                        mm_qt(ps, b, kq, nh, start=False)
                for nh, ps in halves:
                    mm_ctx_all(ps, ctxt_sb, nh)
                osb = out_pool.tile([128, OUT], F16, name="osb", tag="out_sb")
                for nh, ps in halves:
                    nc.scalar.activation(
                        osb[:, _ts(nh, 512)], ps[:],
                        mybir.ActivationFunctionType.Tanh,
                    )
                nc.scalar.dma_start(att_out[b], osb[:])

            def out_and_store_last(b, cx16):
                # nh-outer split: columns [0,512) finish (and tanh+DMA)
                # while the PE is still accumulating columns [512,1024) in
                # the other PSUM tile.
                ps_a = pso_pool.tile([128, 512], F32, name="pso_a", tag="pso_a")
                ps_b = pso_pool.tile([128, 512], F32, name="pso_b", tag="pso_b")
                if with_bias:
                    mm_bias(ps_a, 0)
                for kq in range(KO):
                    mm_qt(ps_a, b, kq, 0, start=(not with_bias and kq == 0))
                ctxt_sb = transpose_ctx(cx16)
                if with_bias:
                    mm_bias(ps_b, 1)
                for kq in range(KO):
                    mm_qt(ps_b, b, kq, 1, start=(not with_bias and kq == 0))
                mm_ctx_all(ps_a, ctxt_sb, 0)
                for q in (0, 1):
                    osb = out_pool.tile([128, 256], F16, name=f"osb{q}",
                                        tag="out_sb")
                    nc.scalar.activation(
                        osb[:], ps_a[:, _ts(q, 256)],
                        mybir.ActivationFunctionType.Tanh,
                    )
                    nc.sync.dma_start(att_out[b][:, _ts(q, 256)], osb[:])
                mm_ctx_all(ps_b, ctxt_sb, 1)
                for q in (0, 1):
                    osb = out_pool.tile([128, 256], F16, name=f"osb{q+2}",
                                        tag="out_sb")
                    nc.scalar.activation(
                        osb[:], ps_b[:, _ts(q, 256)],
                        mybir.ActivationFunctionType.Tanh,
                    )
                    nc.sync.dma_start(att_out[b][:, _ts(q + 2, 256)],
                                       osb[:])

            # ---- Pipelined batch loop ----
            # PE stream per iteration: transp_w(b-1), scores(b), ctx(b-1),
            # out(b-1) [qt-half, ctxT transposes, qt-half, ctx-half].
            # ACT: exp(b), cx casts(b-1), w16(b) + w_out dma, tanh(b-1).
            # DVE: negmax(b), wt casts(b-1), ctxT casts(b-1), recip(b).
            pending = None
            for b in range(B_LOC):
                if b == 1:
                    # scores(1) ahead of transp_w(0): covers the softmax(0)
                    # ACT chain that w16(0) (and thus transp_w(0)) waits on,
                    # so the PE never idles at the qproj->loop seam.
                    pss = scores_mms(b)
                    pb, pw16 = pending
                    wt_sb = transp_w(pb, pw16)
                    ew, ssum = softmax_front(b, pss)
                else:
                    if pending is not None:
                        pb, pw16 = pending
                        wt_sb = transp_w(pb, pw16)
                    pss = scores_mms(b)
                    ew, ssum = softmax_front(b, pss)
                if pending is not None:
                    cx16 = ctx_mms(pb, en_t[pb], wt_sb)
                    part1 = out_part1(pb, cx16)
                # softmax_back emitted after out_part1 so recip(b) doesn't
                # block the ctxT casts at the head of the DVE FIFO.
                w16 = softmax_back(b, ew, ssum)
                if pending is not None:
                    out_part2(pb, *part1)
                pending = (b, w16)
                if b + 2 < B_LOC:
                    load_et(b + 2)
                if b + 1 < B_LOC:
                    load_en(b + 1)
            pb, pw16 = pending
            wt_sb = transp_w(pb, pw16)
            cx16 = ctx_mms(pb, en_t[pb], wt_sb)
            out_and_store_last(pb, cx16)

    nc.compile()
    return nc


def _get_nc(with_bias, slots):
    key = (with_bias, slots)
    if key not in _CACHED:
        _CACHED[key] = _build_program(with_bias, slots)
    return _CACHED[key]


def _prep_inputs(decoder_output, encoder_outputs,
                 W_attn, W_out, b_out, slots, idx_list, assign):
    f16 = np.float16
    kspmax = max(slots)
    spmax = kspmax * 128
    wat_h = W_attn.T.reshape(KO, 128, IN).swapaxes(0, 1).astype(f16)
    # decoder-half tiles first (earliest consumer in the out-projection)
    wot_f = W_out.T.reshape(KC, 128, OUT)
    wot_h = np.concatenate(
        [wot_f[KI:], wot_f[:KI]], axis=0
    ).swapaxes(0, 1).astype(f16)
    bb_h = b_out.reshape(1, OUT).astype(f16)

    in_maps = []
    for c in range(N_CORES):
        gbs = [assign[c][j] for j in range(B_LOC)]
        dec = decoder_output[gbs]         # [8, T, OUT] f32
        enc = encoder_outputs[gbs]        # [8, S, IN] f32
        qt_h = (
            dec.transpose(2, 0, 1).reshape(KO, 128, TALL)
            .swapaxes(0, 1).astype(f16)
        )
        wq_h = np.stack((wat_h, qt_h), axis=2)
        # Compact the encoder to unmasked positions (zero-padded to spmax).
        enc_c = np.zeros((B_LOC, spmax, IN), dtype=np.float32)
        for j, g in enumerate(gbs):
            idx = idx_list[g]
            enc_c[j, : len(idx)] = enc[j, idx]
        et_h = (
            enc_c.transpose(0, 2, 1).reshape(B_LOC, KI, 128, spmax)
            .swapaxes(1, 2).astype(f16)
        )
        en_h = (
            enc_c.reshape(B_LOC, kspmax, 128, IN).swapaxes(1, 2).astype(f16)
        )
        in_maps.append(
            {
                "wq": wq_h,
                "et": et_h,
                "en": en_h,
                "wot": wot_h,
                "bb": bb_h,
            }
        )
    return in_maps


def kernel(decoder_output, encoder_outputs, encoder_padding_mask,
           W_attn, W_out, b_out, _trace=False, _tmpdir=None):
    decoder_output = np.asarray(decoder_output, dtype=np.float32)
    encoder_outputs = np.asarray(encoder_outputs, dtype=np.float32)
    encoder_padding_mask = np.asarray(encoder_padding_mask)
    W_attn = np.asarray(W_attn, dtype=np.float32)
    W_out = np.asarray(W_out, dtype=np.float32)
    b_out = np.asarray(b_out, dtype=np.float32)

    B = decoder_output.shape[0]
    idx_list = [np.flatnonzero(~encoder_padding_mask[b]) for b in range(B)]
    eff = np.array([len(i) for i in idx_list])
    # Deal batches into core-slots sorted by unmasked count: slot j of
    # core c takes rank j*N_CORES + c, so every core shares the same
    # per-slot k-tile profile and later slots compile with fewer tiles.
    order = np.argsort(-eff, kind="stable")
    assign = [[int(order[j * N_CORES + c]) for j in range(B_LOC)]
              for c in range(N_CORES)]
    slots = tuple(
        max(4, -(-int(eff[order[j * N_CORES]]) // 128)) for j in range(B_LOC)
    )

    with_bias = bool(np.any(b_out != 0))
    nc = _get_nc(with_bias, slots)
    in_maps = _prep_inputs(
        decoder_output, encoder_outputs,
        W_attn, W_out, b_out, slots, idx_list, assign,
    )
    kw = {}
    if _trace:
        kw = {"trace": True, "tmpdir": _tmpdir}
    res = run_bass_kernel_spmd(nc, in_maps, core_ids=list(range(N_CORES)), **kw)
    attn_outputs = np.zeros((B, T, OUT), dtype=np.float32)
    attn_weights = np.zeros((B, T, S), dtype=np.float32)
    for c in range(N_CORES):
        att_c = res.results[c]["att_out"].astype(np.float32)
        w_c = res.results[c]["w_out"].astype(np.float32)
        for j in range(B_LOC):
            g = assign[c][j]
            idx = idx_list[g]
            attn_outputs[g] = att_c[j]
            attn_weights[g][:, idx] = w_c[j][:, : len(idx)]
    kernel._last_results = res
    return attn_outputs, attn_weights


# revision 33
# speedup vs baseline: 1.1842x; 1.1842x over previous
"""Trainium2 Bass kernel for the Luong-attention module.

Shapes (hardcoded): B=64, T=128, S=1024, IN=1024, OUT=1024.
Sharding: data-parallel over batch across 8 NeuronCores (8 batches/core).
All matmuls run in fp16 (fp32 PSUM accumulation).

Mask compaction: the padding mask removes ~half the encoder positions.
The host gathers only unmasked positions (padded with zeros to a fixed
SP, a multiple of 128), the kernel computes scores/softmax/ctx over SP
columns, and the host scatters attn_weights back to the full S grid
(masked positions are exactly 0: zeroed E^T columns give score 0, and
exp(0 - max) underflows to 0 in f16 since the unmasked max >> 0).

Per-core dataflow (contraction dim always on partitions):
  q_projT[i,t]   = sum_o W_attnT[o,i] * QT[o,t]          (once, all 8 batches)
  scores[t,s]    = sum_i q_projT[i,t] * ET[i,s]          (s over SP compact)
  softmax along s (free axis): negmax -> Exp(bias)+accum_out -> reciprocal
  w16[t,s]       = ew * (1/sum)   (ACT, per-partition scale)
  wT[s,t]        = PE-transpose(w16)                     (SP/128 tiles)
  ctx[t,i]       = sum_s wT[s,t].T * E[s,i]   (wT stationary, N=512 streams)
  ctxT[i,t]      = PE-transpose(ctx[t,i])                (8 tiles)
  out[t,o]       = tanh(sum_c catT[c,t] * W_outT[c,o] + b_out)
                   with catT k-tiles = [qT; ctxT], W_out host-packed
                   decoder-half first.

Scheduling: phase 0 runs ko-outer over four PSUM groups so the wq DMA
window is filled with real qproj work; the out-projection accumulates
its two 512-col halves in separate PSUM tiles (tanh of one half never
serializes the other half's matmuls); softmax_back is emitted after the
ctxT-cast block so recip never heads-of-line-blocks the DVE FIFO; input
DMAs are deadline-ordered on the sync ring with triple-buffered et/en.
"""

import numpy as np

import concourse.bass as bass
import concourse.mybir as mybir
import concourse.tile as tile
from concourse import bacc
from concourse.bass_utils import run_bass_kernel_spmd
from concourse.masks import make_identity

F16 = mybir.dt.float16
F32 = mybir.dt.float32

N_CORES = 8
B_LOC = 8          # batches per core
T = 128
S = 1024
IN = 1024
OUT = 1024
C = IN + OUT       # concat dim
KO = OUT // 128    # k-tiles over o
KI = IN // 128     # k-tiles over i
KC = C // 128      # k-tiles over c
TALL = B_LOC * T   # stacked t across local batches
SP_MIN = 640       # compacted encoder length (padded, multiple of 128)
N_WARMUP = 54      # identity matmuls: cover the pre-wq DMA window warm

_CACHED = {}


def _ts(i, sz):
    return slice(i * sz, (i + 1) * sz)


def _chunks(total, sz=512):
    """[(offset, len), ...] covering `total` in <=sz pieces."""
    out = []
    off = 0
    while off < total:
        ln = min(sz, total - off)
        out.append((off, ln))
        off += ln
    return out


def _build_program(with_bias, slots):
    # slots[j] = encoder k-tile count for local batch slot j (the host
    # deals batches into slots sorted by unmasked count, so later slots
    # compile with fewer scores/ctx tiles).
    kspmax = max(slots)
    spmax = kspmax * 128
    nc = bacc.Bacc("TRN2", target_bir_lowering=False, debug=False)

    wq = nc.dram_tensor("wq", [128, KO, 2, IN], F16, kind="ExternalInput")
    et = nc.dram_tensor("et", [B_LOC, 128, KI, spmax], F16, kind="ExternalInput")
    en = nc.dram_tensor("en", [B_LOC, 128, kspmax, IN], F16, kind="ExternalInput")
    # wot is host-packed decoder-half first: [:, 0:KO] = W_out[:, IN:C].T
    # tiles, [:, KO:] = W_out[:, 0:IN].T tiles (earliest consumer first).
    wot = nc.dram_tensor("wot", [128, KC, OUT], F16, kind="ExternalInput")
    bb = nc.dram_tensor("bb", [1, OUT], F16, kind="ExternalInput")
    w_out = nc.dram_tensor("w_out", [B_LOC, T, spmax], F16, kind="ExternalOutput")
    att_out = nc.dram_tensor("att_out", [B_LOC, T, OUT], F16, kind="ExternalOutput")

    with tile.TileContext(nc) as tc:
        with (
            tc.tile_pool(name="const", bufs=1) as const_pool,
            tc.tile_pool(name="inp", bufs=3) as in_pool,
            tc.tile_pool(name="ewp", bufs=2) as ew_pool,
            tc.tile_pool(name="statp", bufs=2) as stat_pool,
            tc.tile_pool(name="xp", bufs=2) as x_pool,
            tc.tile_pool(name="outp", bufs=2) as out_pool,
            tc.tile_pool(name="pssp", bufs=2, space="PSUM") as pss_pool,
            tc.tile_pool(name="pmix", bufs=1, space="PSUM") as pmix_pool,
            tc.tile_pool(name="psop", bufs=1, space="PSUM") as pso_pool,
        ):
            ident = const_pool.tile([128, 128], F16)
            make_identity(nc, ident[:])
            ones = const_pool.tile([1, 128], F16)
            nc.vector.memset(ones[:], 1.0)
            # Pre-load the ACT exp/tanh spline tables during the DMA-bound
            # head instead of on exp(0)'s critical path.
            actwarm = const_pool.tile([1, 128], F32)
            nc.scalar.activation(actwarm[:], ones[:],
                                 mybir.ActivationFunctionType.Exp)
            if with_bias:
                bb_sb = const_pool.tile([1, OUT], F16)
                nc.sync.dma_start(bb_sb[:], bb[:])

            # One trigger per ko delivers both wat[ko] and qt[ko].
            wq_sb = const_pool.tile([128, KO, 2, IN], F16)
            for ko in range(KO):
                nc.sync.dma_start(wq_sb[:, ko, :, :], wq[:, ko, :, :])

            qpt_sb = const_pool.tile([128, KI, TALL], F16)

            et_t, en_t = {}, {}

            def load_et(b):
                sp = slots[b] * 128
                et_sb = in_pool.tile([128, KI, sp], F16, name="et")
                nc.sync.dma_start(et_sb[:], et[b][:, :, :sp])
                et_t[b] = et_sb

            def load_en(b):
                ksp = slots[b]
                en_sb = in_pool.tile([128, ksp, IN], F16, name="en")
                nc.sync.dma_start(en_sb[:], en[b][:, :ksp, :])
                en_t[b] = en_sb

            # ---- Phase 0: q_projT[i, t_all] for all local batches ----
            # ko-outer over four PSUM accumulation groups (mi 0-3) keeps
            # the PE ~fully busy with real work while the wq chunks stream
            # in.  mi 4-7 then run as pure compute.
            psqA = pss_pool.tile([128, TALL], F32, name="psqA", tag="pss")
            psqB = pss_pool.tile([128, TALL], F32, name="psqB", tag="pss")
            psqC = pmix_pool.tile([128, TALL], F32, name="psqC", tag="mix")
            # mi=3 splits across the two out-projection PSUM half tiles.
            pso_a = pso_pool.tile([128, 512], F32, name="pso_a", tag="pso_a")
            pso_b = pso_pool.tile([128, 512], F32, name="pso_b", tag="pso_b")

            # Burn the HAM cold window on identity matmuls into psqA's
            # region while the first weight/query DMAs are in flight
            # (qproj's ko=0 start=True overwrites the junk).
            for _ in range(N_WARMUP):
                nc.tensor.matmul(psqA[:, :128], ident[:], ident[:],
                                 start=True, stop=True)

            head = [psqA, psqB, psqC]
            for ko in range(KO):
                for mi in range(3):
                    for nh in range(TALL // 512):
                        nc.tensor.matmul(
                            head[mi][:, _ts(nh, 512)],
                            wq_sb[:, ko, 0, _ts(mi, 128)],
                            wq_sb[:, ko, 1, _ts(nh, 512)],
                            start=(ko == 0),
                            stop=(ko == KO - 1),
                        )
                for nh, pshalf in ((0, pso_a), (1, pso_b)):
                    nc.tensor.matmul(
                        pshalf[:],
                        wq_sb[:, ko, 0, _ts(3, 128)],
                        wq_sb[:, ko, 1, _ts(nh, 512)],
                        start=(ko == 0),
                        stop=(ko == KO - 1),
                    )
            # mi0/mi1 casts first: they gate the pss buffers mi4/mi5 need.
            for mi in range(3):
                nc.vector.tensor_copy(qpt_sb[:, mi, :], head[mi][:])
            nc.vector.tensor_copy(qpt_sb[:, 3, :512], pso_a[:])
            nc.vector.tensor_copy(qpt_sb[:, 3, 512:], pso_b[:])
            for mi in range(4, KI):
                psq = pss_pool.tile([128, TALL], F32, name="psq", tag="pss")
                for ko in range(KO):
                    for nh in range(TALL // 512):
                        nc.tensor.matmul(
                            psq[:, _ts(nh, 512)],
                            wq_sb[:, ko, 0, _ts(mi, 128)],
                            wq_sb[:, ko, 1, _ts(nh, 512)],
                            start=(ko == 0),
                            stop=(ko == KO - 1),
                        )
                nc.vector.tensor_copy(qpt_sb[:, mi, :], psq[:])

            # Deadline-ordered input stream on the sync ring (single FIFO
            # queue): wq, et0, et1, wot-dec, en0, wot-ctx, then per
            # iteration et(b+2), en(b+1).
            load_et(0)
            load_et(1)
            wot_sb = const_pool.tile([128, KC, OUT], F16)
            nc.sync.dma_start(wot_sb[:, :KO, :], wot[:, :KO, :])
            load_en(0)
            nc.sync.dma_start(wot_sb[:, KO:, :], wot[:, KO:, :])

            def scores_mms(b):
                sp = slots[b] * 128
                et_sb = et_t[b]
                pss = pss_pool.tile([128, sp], F32, name="pss", tag="pss")
                for ki in range(KI):
                    for off, ln in _chunks(sp):
                        nc.tensor.matmul(
                            pss[:, off:off + ln],
                            qpt_sb[:, ki, _ts(b, T)],
                            et_sb[:, ki, off:off + ln],
                            start=(ki == 0),
                            stop=(ki == KI - 1),
                        )
                return pss

            def softmax_front(b, pss):
                sp = slots[b] * 128
                negmx = stat_pool.tile([128, 1], F32, name="negmx")
                nc.vector.reduce_max(
                    negmx[:], pss[:], axis=mybir.AxisListType.X, negate=True
                )
                ew = ew_pool.tile([128, sp], F16, name="ew")
                ssum = stat_pool.tile([128, 1], F32, name="ssum")
                nc.scalar.activation(
                    ew[:],
                    pss[:],
                    mybir.ActivationFunctionType.Exp,
                    bias=negmx[:],
                    scale=1.0,
                    accum_out=ssum[:],
                )
                return ew, ssum

            def softmax_back(b, ew, ssum):
                # Normalize on ACT (per-partition 1/sum scale); the
                # normalized w16 feeds both the w_out DMA and the PE
                # transpose, so ctx needs no downstream rescale.
                sp = slots[b] * 128
                rs = stat_pool.tile([128, 1], F32, name="rs")
                nc.vector.reciprocal(rs[:], ssum[:])
                w16 = ew_pool.tile([128, sp], F16, name="w16")
                nc.scalar.mul(w16[:], ew[:], rs[:])
                nc.scalar.dma_start(w_out[b][:, :sp], w16[:])
                return w16

            def transp_w(b, w16):
                # wT[s, t] via PE transpose; the DVE casts drain during the
                # next scores block.
                ksp = slots[b]
                pst = pmix_pool.tile([128, ksp, T], F16, name="pst", tag="mix")
                for st in range(ksp):
                    nc.tensor.matmul(
                        pst[:, st, :],
                        w16[:, _ts(st, 128)],
                        ident[:],
                        is_transpose=True,
                        start=True,
                        stop=True,
                    )
                wt_sb = x_pool.tile([128, ksp, T], F16, name="wt")
                h = ksp // 2
                nc.vector.tensor_copy(wt_sb[:, :h, :], pst[:, :h, :])
                nc.vector.tensor_copy(wt_sb[:, h:, :], pst[:, h:, :])
                return wt_sb

            def ctx_mms(b, en_sb, wt_sb):
                # ctx[t, i] = sum_s w16[t,s] E[s,i]: wT tiles stationary,
                # E streams at N=512.  Weights are already normalized, so
                # the PSUM->SBUF cast is a plain ACT copy.
                ksp = slots[b]
                psc = pmix_pool.tile([128, IN], F32, name="psc", tag="mix")
                for ks in range(ksp):
                    for nh in range(IN // 512):
                        nc.tensor.matmul(
                            psc[:, _ts(nh, 512)],
                            wt_sb[:, ks, :],
                            en_sb[:, ks, _ts(nh, 512)],
                            start=(ks == 0),
                            stop=(ks == ksp - 1),
                        )
                cx16 = x_pool.tile([128, IN], F16, name="cx16")
                nc.scalar.copy(cx16[:, : IN // 2], psc[:, : IN // 2])
                nc.scalar.copy(cx16[:, IN // 2 :], psc[:, IN // 2 :])
                return cx16

            def transpose_ctx(cx16):
                # ctxT[i, t] via PE transpose of cx16 (cheap: ~60ns/tile);
                # the DVE cast of each half chases the transposes.
                pct = pmix_pool.tile([128, KI, T], F16, name="pct", tag="mix")
                ctxt_sb = x_pool.tile([128, KI, T], F16, name="ctxT")
                for j in range(KI):
                    nc.tensor.matmul(
                        pct[:, j, :],
                        cx16[:, _ts(j, 128)],
                        ident[:],
                        is_transpose=True,
                        start=True,
                        stop=True,
                    )
                    if j == KI // 2 - 1:
                        nc.vector.tensor_copy(
                            ctxt_sb[:, : KI // 2, :], pct[:, : KI // 2, :])
                nc.vector.tensor_copy(
                    ctxt_sb[:, KI // 2 :, :], pct[:, KI // 2 :, :])
                return ctxt_sb

            def mm_qt(ps, b, kq, nh, start):
                nc.tensor.matmul(
                    ps[:],
                    wq_sb[:, kq, 1, _ts(b, T)],
                    wot_sb[:, kq, _ts(nh, 512)],
                    start=start,
                    stop=False,
                )

            def mm_bias(ps, nh):
                nc.tensor.matmul(
                    ps[:],
                    ones[:1, :],
                    bb_sb[:1, _ts(nh, 512)],
                    start=True,
                    stop=False,
                )

            def mm_ctx_all(ps, ctxt_sb, nh):
                for kc in range(KI):
                    nc.tensor.matmul(
                        ps[:],
                        ctxt_sb[:, kc, :],
                        wot_sb[:, KO + kc, _ts(nh, 512)],
                        start=False,
                        stop=(kc == KI - 1),
                    )

            def out_part1(b, cx16):
                # out[t, o] = tanh(catT.T @ W_outT + b_out).  The two
                # 512-col halves accumulate in separate PSUM tiles so the
                # tanh reads never serialize the other half's matmuls.
                ps_a = pso_pool.tile([128, 512], F32, name="pso_a", tag="pso_a")
                ps_b = pso_pool.tile([128, 512], F32, name="pso_b", tag="pso_b")
                halves = ((0, ps_a), (1, ps_b))
                if with_bias:
                    for nh, ps in halves:
                        mm_bias(ps, nh)
                for kq in range(KO // 2):
                    for nh, ps in halves:
                        mm_qt(ps, b, kq, nh, start=(not with_bias and kq == 0))
                ctxt_sb = transpose_ctx(cx16)
                return ps_a, ps_b, ctxt_sb

            def out_part2(b, ps_a, ps_b, ctxt_sb):
                halves = ((0, ps_a), (1, ps_b))
                for kq in range(KO // 2, KO):
                    for nh, ps in halves:
                        mm_qt(ps, b, kq, nh, start=False)
                for nh, ps in halves:
                    mm_ctx_all(ps, ctxt_sb, nh)
                osb = out_pool.tile([128, OUT], F16, name="osb", tag="out_sb")
                for nh, ps in halves:
                    nc.scalar.activation(
                        osb[:, _ts(nh, 512)], ps[:],
                        mybir.ActivationFunctionType.Tanh,
                    )
                nc.scalar.dma_start(att_out[b], osb[:])

            def out_and_store_last(b, cx16):
                # nh-outer split: columns [0,512) finish (and tanh+DMA)
                # while the PE is still accumulating columns [512,1024) in
                # the other PSUM tile.
                ps_a = pso_pool.tile([128, 512], F32, name="pso_a", tag="pso_a")
                ps_b = pso_pool.tile([128, 512], F32, name="pso_b", tag="pso_b")
                if with_bias:
                    mm_bias(ps_a, 0)
                for kq in range(KO):
                    mm_qt(ps_a, b, kq, 0, start=(not with_bias and kq == 0))
                ctxt_sb = transpose_ctx(cx16)
                if with_bias:
                    mm_bias(ps_b, 1)
                for kq in range(KO):
                    mm_qt(ps_b, b, kq, 1, start=(not with_bias and kq == 0))
                mm_ctx_all(ps_a, ctxt_sb, 0)
                for q in (0, 1):
                    osb = out_pool.tile([128, 256], F16, name=f"osb{q}",
                                        tag="out_sb")
                    nc.scalar.activation(
                        osb[:], ps_a[:, _ts(q, 256)],
                        mybir.ActivationFunctionType.Tanh,
                    )
                    nc.sync.dma_start(att_out[b][:, _ts(q, 256)], osb[:])
                mm_ctx_all(ps_b, ctxt_sb, 1)
                for q in (0, 1):
                    osb = out_pool.tile([128, 256], F16, name=f"osb{q+2}",
                                        tag="out_sb")
                    nc.scalar.activation(
                        osb[:], ps_b[:, _ts(q, 256)],
                        mybir.ActivationFunctionType.Tanh,
                    )
                    nc.sync.dma_start(att_out[b][:, _ts(q + 2, 256)],
                                       osb[:])

            # ---- Pipelined batch loop ----
            # PE stream per iteration: transp_w(b-1), scores(b), ctx(b-1),
            # out(b-1) [qt-half, ctxT transposes, qt-half, ctx-half].
            # ACT: exp(b), cx casts(b-1), w16(b) + w_out dma, tanh(b-1).
            # DVE: negmax(b), wt casts(b-1), ctxT casts(b-1), recip(b).
            pending = None
            for b in range(B_LOC):
                if b == 1:
                    # scores(1) ahead of transp_w(0): covers the softmax(0)
                    # ACT chain that w16(0) (and thus transp_w(0)) waits on,
                    # so the PE never idles at the qproj->loop seam.
                    pss = scores_mms(b)
                    pb, pw16 = pending
                    wt_sb = transp_w(pb, pw16)
                    ew, ssum = softmax_front(b, pss)
                else:
                    if pending is not None:
                        pb, pw16 = pending
                        wt_sb = transp_w(pb, pw16)
                    pss = scores_mms(b)
                    ew, ssum = softmax_front(b, pss)
                if pending is not None:
                    cx16 = ctx_mms(pb, en_t[pb], wt_sb)
                    part1 = out_part1(pb, cx16)
                # softmax_back emitted after out_part1 so recip(b) doesn't
                # block the ctxT casts at the head of the DVE FIFO.
                w16 = softmax_back(b, ew, ssum)
                if pending is not None:
                    out_part2(pb, *part1)
                pending = (b, w16)
                if b + 2 < B_LOC:
                    load_et(b + 2)
                if b + 1 < B_LOC:
                    load_en(b + 1)
            pb, pw16 = pending
            wt_sb = transp_w(pb, pw16)
            cx16 = ctx_mms(pb, en_t[pb], wt_sb)
            out_and_store_last(pb, cx16)

    nc.compile()
    return nc


def _get_nc(with_bias, slots):
    key = (with_bias, slots)
    if key not in _CACHED:
        _CACHED[key] = _build_program(with_bias, slots)
    return _CACHED[key]


def _prep_inputs(decoder_output, encoder_outputs,
                 W_attn, W_out, b_out, slots, idx_list, assign):
    f16 = np.float16
    kspmax = max(slots)
    spmax = kspmax * 128
    wat_h = W_attn.T.reshape(KO, 128, IN).swapaxes(0, 1).astype(f16)
    # decoder-half tiles first (earliest consumer in the out-projection)
    wot_f = W_out.T.reshape(KC, 128, OUT)
    wot_h = np.concatenate(
        [wot_f[KI:], wot_f[:KI]], axis=0
    ).swapaxes(0, 1).astype(f16)
    bb_h = b_out.reshape(1, OUT).astype(f16)

    in_maps = []
    for c in range(N_CORES):
        gbs = [assign[c][j] for j in range(B_LOC)]
        dec = decoder_output[gbs]         # [8, T, OUT] f32
        enc = encoder_outputs[gbs]        # [8, S, IN] f32
        qt_h = (
            dec.transpose(2, 0, 1).reshape(KO, 128, TALL)
            .swapaxes(0, 1).astype(f16)
        )
        wq_h = np.stack((wat_h, qt_h), axis=2)
        # Compact the encoder to unmasked positions (zero-padded to spmax).
        enc_c = np.zeros((B_LOC, spmax, IN), dtype=np.float32)
        for j, g in enumerate(gbs):
            idx = idx_list[g]
            enc_c[j, : len(idx)] = enc[j, idx]
        et_h = (
            enc_c.transpose(0, 2, 1).reshape(B_LOC, KI, 128, spmax)
            .swapaxes(1, 2).astype(f16)
        )
        en_h = (
            enc_c.reshape(B_LOC, kspmax, 128, IN).swapaxes(1, 2).astype(f16)
        )
        in_maps.append(
            {
                "wq": wq_h,
                "et": et_h,
                "en": en_h,
                "wot": wot_h,
                "bb": bb_h,
            }
        )
    return in_maps


def kernel(decoder_output, encoder_outputs, encoder_padding_mask,
           W_attn, W_out, b_out, _trace=False, _tmpdir=None):
    decoder_output = np.asarray(decoder_output, dtype=np.float32)
    encoder_outputs = np.asarray(encoder_outputs, dtype=np.float32)
    encoder_padding_mask = np.asarray(encoder_padding_mask)
    W_attn = np.asarray(W_attn, dtype=np.float32)
    W_out = np.asarray(W_out, dtype=np.float32)
    b_out = np.asarray(b_out, dtype=np.float32)

    B = decoder_output.shape[0]
    idx_list = [np.flatnonzero(~encoder_padding_mask[b]) for b in range(B)]
    eff = np.array([len(i) for i in idx_list])
    # Deal batches into core-slots sorted by unmasked count: slot j of
    # core c takes rank j*N_CORES + c, so every core shares the same
    # per-slot k-tile profile and later slots compile with fewer tiles.
    order = np.argsort(-eff, kind="stable")
    assign = [[int(order[j * N_CORES + c]) for j in range(B_LOC)]
              for c in range(N_CORES)]
    slots = tuple(
        max(4, -(-int(eff[order[j * N_CORES]]) // 128)) for j in range(B_LOC)
    )

    with_bias = bool(np.any(b_out != 0))
    nc = _get_nc(with_bias, slots)
    in_maps = _prep_inputs(
        decoder_output, encoder_outputs,
        W_attn, W_out, b_out, slots, idx_list, assign,
    )
    kw = {}
    if _trace:
        kw = {"trace": True, "tmpdir": _tmpdir}
    res = run_bass_kernel_spmd(nc, in_maps, core_ids=list(range(N_CORES)), **kw)
    attn_outputs = np.zeros((B, T, OUT), dtype=np.float32)
    attn_weights = np.zeros((B, T, S), dtype=np.float32)
    for c in range(N_CORES):
        att_c = res.results[c]["att_out"].astype(np.float32)
        w_c = res.results[c]["w_out"].astype(np.float32)
        for j in range(B_LOC):
            g = assign[c][j]
            idx = idx_list[g]
            attn_outputs[g] = att_c[j]
            attn_weights[g][:, idx] = w_c[j][:, : len(idx)]
    kernel._last_results = res
    return attn_outputs, attn_weights


# revision 34
# speedup vs baseline: 1.1844x; 1.0002x over previous
"""Trainium2 Bass kernel for the Luong-attention module.

Shapes (hardcoded): B=64, T=128, S=1024, IN=1024, OUT=1024.
Sharding: data-parallel over batch across 8 NeuronCores (8 batches/core).
All matmuls run in fp16 (fp32 PSUM accumulation).

Mask compaction: the padding mask removes ~half the encoder positions.
The host gathers only unmasked positions (padded with zeros to a fixed
SP, a multiple of 128), the kernel computes scores/softmax/ctx over SP
columns, and the host scatters attn_weights back to the full S grid
(masked positions are exactly 0: zeroed E^T columns give score 0, and
exp(0 - max) underflows to 0 in f16 since the unmasked max >> 0).

Per-core dataflow (contraction dim always on partitions):
  q_projT[i,t]   = sum_o W_attnT[o,i] * QT[o,t]          (once, all 8 batches)
  scores[t,s]    = sum_i q_projT[i,t] * ET[i,s]          (s over SP compact)
  softmax along s (free axis): negmax -> Exp(bias)+accum_out -> reciprocal
  w16[t,s]       = ew * (1/sum)   (ACT, per-partition scale)
  wT[s,t]        = PE-transpose(w16)                     (SP/128 tiles)
  ctx[t,i]       = sum_s wT[s,t].T * E[s,i]   (wT stationary, N=512 streams)
  ctxT[i,t]      = PE-transpose(ctx[t,i])                (8 tiles)
  out[t,o]       = tanh(sum_c catT[c,t] * W_outT[c,o] + b_out)
                   with catT k-tiles = [qT; ctxT], W_out host-packed
                   decoder-half first.

Scheduling: phase 0 runs ko-outer over four PSUM groups so the wq DMA
window is filled with real qproj work; the out-projection accumulates
its two 512-col halves in separate PSUM tiles (tanh of one half never
serializes the other half's matmuls); softmax_back is emitted after the
ctxT-cast block so recip never heads-of-line-blocks the DVE FIFO; input
DMAs are deadline-ordered on the sync ring with triple-buffered et/en.
"""

import numpy as np

import concourse.bass as bass
import concourse.mybir as mybir
import concourse.tile as tile
from concourse import bacc
from concourse.bass_utils import run_bass_kernel_spmd
from concourse.masks import make_identity

F16 = mybir.dt.float16
F32 = mybir.dt.float32

N_CORES = 8
B_LOC = 8          # batches per core
T = 128
S = 1024
IN = 1024
OUT = 1024
C = IN + OUT       # concat dim
KO = OUT // 128    # k-tiles over o
KI = IN // 128     # k-tiles over i
KC = C // 128      # k-tiles over c
TALL = B_LOC * T   # stacked t across local batches
SP_MIN = 640       # compacted encoder length (padded, multiple of 128)
N_WARMUP = 54      # identity matmuls: cover the pre-wq DMA window warm

_CACHED = {}


def _ts(i, sz):
    return slice(i * sz, (i + 1) * sz)


def _chunks(total, sz=512):
    """[(offset, len), ...] covering `total` in <=sz pieces."""
    out = []
    off = 0
    while off < total:
        ln = min(sz, total - off)
        out.append((off, ln))
        off += ln
    return out


def _build_program(with_bias, slots):
    # slots[j] = encoder k-tile count for local batch slot j (the host
    # deals batches into slots sorted by unmasked count, so later slots
    # compile with fewer scores/ctx tiles).
    kspmax = max(slots)
    spmax = kspmax * 128
    nc = bacc.Bacc("TRN2", target_bir_lowering=False, debug=False)

    wq = nc.dram_tensor("wq", [128, KO, 2, IN], F16, kind="ExternalInput")
    et = nc.dram_tensor("et", [B_LOC, 128, KI, spmax], F16, kind="ExternalInput")
    en = nc.dram_tensor("en", [B_LOC, 128, kspmax, IN], F16, kind="ExternalInput")
    # wot is host-packed decoder-half first: [:, 0:KO] = W_out[:, IN:C].T
    # tiles, [:, KO:] = W_out[:, 0:IN].T tiles (earliest consumer first).
    wot = nc.dram_tensor("wot", [128, KC, OUT], F16, kind="ExternalInput")
    bb = nc.dram_tensor("bb", [1, OUT], F16, kind="ExternalInput")
    w_out = nc.dram_tensor("w_out", [B_LOC, T, spmax], F16, kind="ExternalOutput")
    att_out = nc.dram_tensor("att_out", [B_LOC, T, OUT], F16, kind="ExternalOutput")

    with tile.TileContext(nc) as tc:
        with (
            tc.tile_pool(name="const", bufs=1) as const_pool,
            tc.tile_pool(name="inp", bufs=3) as in_pool,
            tc.tile_pool(name="ewp", bufs=2) as ew_pool,
            tc.tile_pool(name="statp", bufs=2) as stat_pool,
            tc.tile_pool(name="xp", bufs=2) as x_pool,
            tc.tile_pool(name="outp", bufs=2) as out_pool,
            tc.tile_pool(name="pssp", bufs=2, space="PSUM") as pss_pool,
            tc.tile_pool(name="pmix", bufs=1, space="PSUM") as pmix_pool,
            tc.tile_pool(name="psop", bufs=1, space="PSUM") as pso_pool,
        ):
            ident = const_pool.tile([128, 128], F16)
            make_identity(nc, ident[:])
            ones = const_pool.tile([1, 128], F16)
            nc.vector.memset(ones[:], 1.0)
            # Pre-load the ACT exp/tanh spline tables during the DMA-bound
            # head instead of on exp(0)'s critical path.
            actwarm = const_pool.tile([1, 128], F32)
            nc.scalar.activation(actwarm[:], ones[:],
                                 mybir.ActivationFunctionType.Exp)
            if with_bias:
                bb_sb = const_pool.tile([1, OUT], F16)
                nc.sync.dma_start(bb_sb[:], bb[:])

            # One trigger per ko delivers both wat[ko] and qt[ko].
            wq_sb = const_pool.tile([128, KO, 2, IN], F16)
            for ko in range(KO):
                nc.sync.dma_start(wq_sb[:, ko, :, :], wq[:, ko, :, :])

            qpt_sb = const_pool.tile([128, KI, TALL], F16)

            et_t, en_t = {}, {}

            def load_et(b):
                sp = slots[b] * 128
                et_sb = in_pool.tile([128, KI, sp], F16, name="et")
                nc.sync.dma_start(et_sb[:], et[b][:, :, :sp])
                et_t[b] = et_sb

            def load_en(b):
                ksp = slots[b]
                en_sb = in_pool.tile([128, ksp, IN], F16, name="en")
                nc.sync.dma_start(en_sb[:], en[b][:, :ksp, :])
                en_t[b] = en_sb

            # ---- Phase 0: q_projT[i, t_all] for all local batches ----
            # ko-outer over four PSUM accumulation groups (mi 0-3) keeps
            # the PE ~fully busy with real work while the wq chunks stream
            # in.  mi 4-7 then run as pure compute.
            psqA = pss_pool.tile([128, TALL], F32, name="psqA", tag="pss")
            psqB = pss_pool.tile([128, TALL], F32, name="psqB", tag="pss")
            psqC = pmix_pool.tile([128, TALL], F32, name="psqC", tag="mix")
            # mi=3 splits across the two out-projection PSUM half tiles.
            pso_a = pso_pool.tile([128, 512], F32, name="pso_a", tag="pso_a")
            pso_b = pso_pool.tile([128, 512], F32, name="pso_b", tag="pso_b")

            # Burn the HAM cold window on identity matmuls into psqA's
            # region while the first weight/query DMAs are in flight
            # (qproj's ko=0 start=True overwrites the junk).
            for _ in range(N_WARMUP):
                nc.tensor.matmul(psqA[:, :128], ident[:], ident[:],
                                 start=True, stop=True)

            head = [psqA, psqB, psqC]
            for ko in range(KO):
                for mi in range(3):
                    for nh in range(TALL // 512):
                        nc.tensor.matmul(
                            head[mi][:, _ts(nh, 512)],
                            wq_sb[:, ko, 0, _ts(mi, 128)],
                            wq_sb[:, ko, 1, _ts(nh, 512)],
                            start=(ko == 0),
                            stop=(ko == KO - 1),
                        )
                for nh, pshalf in ((0, pso_a), (1, pso_b)):
                    nc.tensor.matmul(
                        pshalf[:],
                        wq_sb[:, ko, 0, _ts(3, 128)],
                        wq_sb[:, ko, 1, _ts(nh, 512)],
                        start=(ko == 0),
                        stop=(ko == KO - 1),
                    )
            # mi0/mi1 casts first: they gate the pss buffers mi4/mi5 need.
            for mi in range(3):
                nc.vector.tensor_copy(qpt_sb[:, mi, :], head[mi][:])
            nc.vector.tensor_copy(qpt_sb[:, 3, :512], pso_a[:])
            nc.vector.tensor_copy(qpt_sb[:, 3, 512:], pso_b[:])
            for mi in range(4, KI):
                psq = pss_pool.tile([128, TALL], F32, name="psq", tag="pss")
                for ko in range(KO):
                    for nh in range(TALL // 512):
                        nc.tensor.matmul(
                            psq[:, _ts(nh, 512)],
                            wq_sb[:, ko, 0, _ts(mi, 128)],
                            wq_sb[:, ko, 1, _ts(nh, 512)],
                            start=(ko == 0),
                            stop=(ko == KO - 1),
                        )
                nc.vector.tensor_copy(qpt_sb[:, mi, :], psq[:])

            # Deadline-ordered input stream on the sync ring (single FIFO
            # queue): wq, et0, et1, wot-dec, en0, wot-ctx, then per
            # iteration et(b+2), en(b+1).
            load_et(0)
            load_et(1)
            wot_sb = const_pool.tile([128, KC, OUT], F16)
            nc.sync.dma_start(wot_sb[:, :KO, :], wot[:, :KO, :])
            load_en(0)
            nc.sync.dma_start(wot_sb[:, KO:, :], wot[:, KO:, :])

            def scores_mms(b):
                sp = slots[b] * 128
                et_sb = et_t[b]
                pss = pss_pool.tile([128, sp], F32, name="pss", tag="pss")
                for ki in range(KI):
                    for off, ln in _chunks(sp):
                        nc.tensor.matmul(
                            pss[:, off:off + ln],
                            qpt_sb[:, ki, _ts(b, T)],
                            et_sb[:, ki, off:off + ln],
                            start=(ki == 0),
                            stop=(ki == KI - 1),
                        )
                return pss

            def softmax_front(b, pss):
                sp = slots[b] * 128
                negmx = stat_pool.tile([128, 1], F32, name="negmx")
                nc.vector.reduce_max(
                    negmx[:], pss[:], axis=mybir.AxisListType.X, negate=True
                )
                ew = ew_pool.tile([128, sp], F16, name="ew")
                ssum = stat_pool.tile([128, 1], F32, name="ssum")
                nc.scalar.activation(
                    ew[:],
                    pss[:],
                    mybir.ActivationFunctionType.Exp,
                    bias=negmx[:],
                    scale=1.0,
                    accum_out=ssum[:],
                )
                return ew, ssum

            def softmax_back(b, ew, ssum):
                # Normalize on ACT (per-partition 1/sum scale); the
                # normalized w16 feeds both the w_out DMA and the PE
                # transpose, so ctx needs no downstream rescale.
                sp = slots[b] * 128
                rs = stat_pool.tile([128, 1], F32, name="rs")
                nc.vector.reciprocal(rs[:], ssum[:])
                w16 = ew_pool.tile([128, sp], F16, name="w16")
                nc.scalar.mul(w16[:], ew[:], rs[:])
                nc.scalar.dma_start(w_out[b][:, :sp], w16[:])
                return w16

            def transp_w(b, w16):
                # wT[s, t] via PE transpose; the DVE casts drain during the
                # next scores block.
                ksp = slots[b]
                pst = pmix_pool.tile([128, ksp, T], F16, name="pst", tag="mix")
                for st in range(ksp):
                    nc.tensor.matmul(
                        pst[:, st, :],
                        w16[:, _ts(st, 128)],
                        ident[:],
                        is_transpose=True,
                        start=True,
                        stop=True,
                    )
                wt_sb = x_pool.tile([128, ksp, T], F16, name="wt")
                h = ksp // 2
                nc.vector.tensor_copy(wt_sb[:, :h, :], pst[:, :h, :])
                nc.vector.tensor_copy(wt_sb[:, h:, :], pst[:, h:, :])
                return wt_sb

            def ctx_mms(b, en_sb, wt_sb):
                # ctx[t, i] = sum_s w16[t,s] E[s,i]: wT tiles stationary,
                # E streams at N=512.  Weights are already normalized, so
                # the PSUM->SBUF cast is a plain ACT copy.
                ksp = slots[b]
                psc = pmix_pool.tile([128, IN], F32, name="psc", tag="mix")
                for ks in range(ksp):
                    for nh in range(IN // 512):
                        nc.tensor.matmul(
                            psc[:, _ts(nh, 512)],
                            wt_sb[:, ks, :],
                            en_sb[:, ks, _ts(nh, 512)],
                            start=(ks == 0),
                            stop=(ks == ksp - 1),
                        )
                cx16 = x_pool.tile([128, IN], F16, name="cx16")
                nc.scalar.copy(cx16[:, : IN // 2], psc[:, : IN // 2])
                nc.scalar.copy(cx16[:, IN // 2 :], psc[:, IN // 2 :])
                return cx16

            def transpose_ctx(cx16):
                # ctxT[i, t] via PE transpose of cx16 (cheap: ~60ns/tile);
                # the DVE cast of each half chases the transposes.
                pct = pmix_pool.tile([128, KI, T], F16, name="pct", tag="mix")
                ctxt_sb = x_pool.tile([128, KI, T], F16, name="ctxT")
                for j in range(KI):
                    nc.tensor.matmul(
                        pct[:, j, :],
                        cx16[:, _ts(j, 128)],
                        ident[:],
                        is_transpose=True,
                        start=True,
                        stop=True,
                    )
                    if j == KI // 2 - 1:
                        nc.vector.tensor_copy(
                            ctxt_sb[:, : KI // 2, :], pct[:, : KI // 2, :])
                nc.vector.tensor_copy(
                    ctxt_sb[:, KI // 2 :, :], pct[:, KI // 2 :, :])
                return ctxt_sb

            def mm_qt(ps, b, kq, nh, start):
                nc.tensor.matmul(
                    ps[:],
                    wq_sb[:, kq, 1, _ts(b, T)],
                    wot_sb[:, kq, _ts(nh, 512)],
                    start=start,
                    stop=False,
                )

            def mm_bias(ps, nh):
                nc.tensor.matmul(
                    ps[:],
                    ones[:1, :],
                    bb_sb[:1, _ts(nh, 512)],
                    start=True,
                    stop=False,
                )

            def mm_ctx_all(ps, ctxt_sb, nh):
                for kc in range(KI):
                    nc.tensor.matmul(
                        ps[:],
                        ctxt_sb[:, kc, :],
                        wot_sb[:, KO + kc, _ts(nh, 512)],
                        start=False,
                        stop=(kc == KI - 1),
                    )

            def out_part1(b, cx16):
                # out[t, o] = tanh(catT.T @ W_outT + b_out).  The two
                # 512-col halves accumulate in separate PSUM tiles so the
                # tanh reads never serialize the other half's matmuls.
                ps_a = pso_pool.tile([128, 512], F32, name="pso_a", tag="pso_a")
                ps_b = pso_pool.tile([128, 512], F32, name="pso_b", tag="pso_b")
                halves = ((0, ps_a), (1, ps_b))
                if with_bias:
                    for nh, ps in halves:
                        mm_bias(ps, nh)
                for kq in range(KO // 2):
                    for nh, ps in halves:
                        mm_qt(ps, b, kq, nh, start=(not with_bias and kq == 0))
                ctxt_sb = transpose_ctx(cx16)
                return ps_a, ps_b, ctxt_sb

            def out_part2(b, ps_a, ps_b, ctxt_sb):
                halves = ((0, ps_a), (1, ps_b))
                for kq in range(KO // 2, KO):
                    for nh, ps in halves:
                        mm_qt(ps, b, kq, nh, start=False)
                for nh, ps in halves:
                    mm_ctx_all(ps, ctxt_sb, nh)
                osb = out_pool.tile([128, OUT], F16, name="osb", tag="out_sb")
                for nh, ps in halves:
                    nc.scalar.activation(
                        osb[:, _ts(nh, 512)], ps[:],
                        mybir.ActivationFunctionType.Tanh,
                    )
                nc.scalar.dma_start(att_out[b], osb[:])

            def out_and_store_last(b, cx16):
                # nh-outer split: columns [0,512) finish (and tanh+DMA)
                # while the PE is still accumulating columns [512,1024) in
                # the other PSUM tile.
                ps_a = pso_pool.tile([128, 512], F32, name="pso_a", tag="pso_a")
                ps_b = pso_pool.tile([128, 512], F32, name="pso_b", tag="pso_b")
                if with_bias:
                    mm_bias(ps_a, 0)
                for kq in range(KO):
                    mm_qt(ps_a, b, kq, 0, start=(not with_bias and kq == 0))
                ctxt_sb = transpose_ctx(cx16)
                if with_bias:
                    mm_bias(ps_b, 1)
                for kq in range(KO):
                    mm_qt(ps_b, b, kq, 1, start=(not with_bias and kq == 0))
                mm_ctx_all(ps_a, ctxt_sb, 0)
                for q in (0, 1):
                    osb = out_pool.tile([128, 256], F16, name=f"osb{q}",
                                        tag="out_sb")
                    nc.scalar.activation(
                        osb[:], ps_a[:, _ts(q, 256)],
                        mybir.ActivationFunctionType.Tanh,
                    )
                    nc.sync.dma_start(att_out[b][:, _ts(q, 256)], osb[:])
                mm_ctx_all(ps_b, ctxt_sb, 1)
                # single tanh + single sync-ring DMA: shortest serial chain
                # after the last matmul (the DMA issue runs parallel on the
                # idle sync sequencer).
                osb = out_pool.tile([128, 512], F16, name="osb_h1",
                                    tag="out_sb")
                nc.scalar.activation(
                    osb[:], ps_b[:],
                    mybir.ActivationFunctionType.Tanh,
                )
                nc.sync.dma_start(att_out[b][:, 512:], osb[:])

            # ---- Pipelined batch loop ----
            # PE stream per iteration: transp_w(b-1), scores(b), ctx(b-1),
            # out(b-1) [qt-half, ctxT transposes, qt-half, ctx-half].
            # ACT: exp(b), cx casts(b-1), w16(b) + w_out dma, tanh(b-1).
            # DVE: negmax(b), wt casts(b-1), ctxT casts(b-1), recip(b).
            pending = None
            for b in range(B_LOC):
                if b == 1:
                    # scores(1) ahead of transp_w(0): covers the softmax(0)
                    # ACT chain that w16(0) (and thus transp_w(0)) waits on,
                    # so the PE never idles at the qproj->loop seam.
                    pss = scores_mms(b)
                    pb, pw16 = pending
                    wt_sb = transp_w(pb, pw16)
                    ew, ssum = softmax_front(b, pss)
                else:
                    if pending is not None:
                        pb, pw16 = pending
                        wt_sb = transp_w(pb, pw16)
                    pss = scores_mms(b)
                    ew, ssum = softmax_front(b, pss)
                if pending is not None:
                    cx16 = ctx_mms(pb, en_t[pb], wt_sb)
                    part1 = out_part1(pb, cx16)
                # softmax_back emitted after out_part1 so recip(b) doesn't
                # block the ctxT casts at the head of the DVE FIFO.
                w16 = softmax_back(b, ew, ssum)
                if pending is not None:
                    out_part2(pb, *part1)
                pending = (b, w16)
                if b + 2 < B_LOC:
                    load_et(b + 2)
                if b + 1 < B_LOC:
                    load_en(b + 1)
            pb, pw16 = pending
            wt_sb = transp_w(pb, pw16)
            cx16 = ctx_mms(pb, en_t[pb], wt_sb)
            out_and_store_last(pb, cx16)

    nc.compile()
    return nc


def _get_nc(with_bias, slots):
    key = (with_bias, slots)
    if key not in _CACHED:
        _CACHED[key] = _build_program(with_bias, slots)
    return _CACHED[key]


def _prep_inputs(decoder_output, encoder_outputs,
                 W_attn, W_out, b_out, slots, idx_list, assign):
    f16 = np.float16
    kspmax = max(slots)
    spmax = kspmax * 128
    wat_h = W_attn.T.reshape(KO, 128, IN).swapaxes(0, 1).astype(f16)
    # decoder-half tiles first (earliest consumer in the out-projection)
    wot_f = W_out.T.reshape(KC, 128, OUT)
    wot_h = np.concatenate(
        [wot_f[KI:], wot_f[:KI]], axis=0
    ).swapaxes(0, 1).astype(f16)
    bb_h = b_out.reshape(1, OUT).astype(f16)

    in_maps = []
    for c in range(N_CORES):
        gbs = [assign[c][j] for j in range(B_LOC)]
        dec = decoder_output[gbs]         # [8, T, OUT] f32
        enc = encoder_outputs[gbs]        # [8, S, IN] f32
        qt_h = (
            dec.transpose(2, 0, 1).reshape(KO, 128, TALL)
            .swapaxes(0, 1).astype(f16)
        )
        wq_h = np.stack((wat_h, qt_h), axis=2)
        # Compact the encoder to unmasked positions (zero-padded to spmax).
        enc_c = np.zeros((B_LOC, spmax, IN), dtype=np.float32)
        for j, g in enumerate(gbs):
            idx = idx_list[g]
            enc_c[j, : len(idx)] = enc[j, idx]
        et_h = (
            enc_c.transpose(0, 2, 1).reshape(B_LOC, KI, 128, spmax)
            .swapaxes(1, 2).astype(f16)
        )
        en_h = (
            enc_c.reshape(B_LOC, kspmax, 128, IN).swapaxes(1, 2).astype(f16)
        )
        in_maps.append(
            {
                "wq": wq_h,
                "et": et_h,
                "en": en_h,
                "wot": wot_h,
                "bb": bb_h,
            }
        )
    return in_maps


def kernel(decoder_output, encoder_outputs, encoder_padding_mask,
           W_attn, W_out, b_out, _trace=False, _tmpdir=None):
    decoder_output = np.asarray(decoder_output, dtype=np.float32)
    encoder_outputs = np.asarray(encoder_outputs, dtype=np.float32)
    encoder_padding_mask = np.asarray(encoder_padding_mask)
    W_attn = np.asarray(W_attn, dtype=np.float32)
    W_out = np.asarray(W_out, dtype=np.float32)
    b_out = np.asarray(b_out, dtype=np.float32)

    B = decoder_output.shape[0]
    idx_list = [np.flatnonzero(~encoder_padding_mask[b]) for b in range(B)]
    eff = np.array([len(i) for i in idx_list])
    # Deal batches into core-slots sorted by unmasked count: slot j of
    # core c takes rank j*N_CORES + c, so every core shares the same
    # per-slot k-tile profile and later slots compile with fewer tiles.
    order = np.argsort(-eff, kind="stable")
    assign = [[int(order[j * N_CORES + c]) for j in range(B_LOC)]
              for c in range(N_CORES)]
    slots = tuple(
        max(4, -(-int(eff[order[j * N_CORES]]) // 128)) for j in range(B_LOC)
    )

    with_bias = bool(np.any(b_out != 0))
    nc = _get_nc(with_bias, slots)
    in_maps = _prep_inputs(
        decoder_output, encoder_outputs,
        W_attn, W_out, b_out, slots, idx_list, assign,
    )
    kw = {}
    if _trace:
        kw = {"trace": True, "tmpdir": _tmpdir}
    res = run_bass_kernel_spmd(nc, in_maps, core_ids=list(range(N_CORES)), **kw)
    attn_outputs = np.zeros((B, T, OUT), dtype=np.float32)
    attn_weights = np.zeros((B, T, S), dtype=np.float32)
    for c in range(N_CORES):
        att_c = res.results[c]["att_out"].astype(np.float32)
        w_c = res.results[c]["w_out"].astype(np.float32)
        for j in range(B_LOC):
            g = assign[c][j]
            idx = idx_list[g]
            attn_outputs[g] = att_c[j]
            attn_weights[g][:, idx] = w_c[j][:, : len(idx)]
    kernel._last_results = res
    return attn_outputs, attn_weights
